# revision 19
# baseline (speedup 1.0000x reference)
"""BERT self-attention on 8 Trainium2 NeuronCores.

Sharding: data-parallel over batch (B=8 -> 1 batch element per core).
Every core runs the same single-core Bass kernel on its own batch slice;
weights/mask are replicated. The final output is a host-side stack.

Per-core algorithm (S=1024, HID=1024, NH=16, HD=64), all matmuls bf16
with fp32 PSUM accumulation:

  xT = X^T (host-transposed, bf16)             [HID, S]
  Q^T = Wq^T @ X^T   (lhsT = Wq natural)       [HID, S]  (+bq per-partition)
  K^T = Wk^T @ X^T                             [HID, S]  (+bk per-partition)
  V   = X @ Wv       (lhsT = xT)               [S, HID]  (+bv broadcast; see below)
  per head h:
    S^T = K_h @ Q_h^T            (scoresT: [k, q]; K^T stored zero-padded to
                                  128 contraction rows so FWL stays enabled)
    P^T = exp(S^T/8 + mask[k])   (ScalarE, mask is per-partition in this layout,
                                  max-subtraction skipped: |scores/8| <~ 4)
    ctx = P^T.T @ [V_h + bv | 1] (lhsT = P^T directly, no transposes anywhere;
                                  the ones column yields the softmax denominator Z)
    out[:, h] = ctx[:, :64] * (1/Z)   (== softmax(S) @ (V+bv) = attn + bv)

  Measured on 8 axon TRN2 cores: ~215 us/core HW exec, rel err 3.5e-3 vs
  the fp32 reference (all error from bf16 matmul operands).
"""

import functools

import numpy as np
import ml_dtypes

B, S, HID = 8, 1024, 1024
NH, HD = 16, 64
P = 128
NCH = HID // P  # hid chunks (8)
NKT = S // P  # key tiles (8)
NQT = S // P  # query tiles (8)
VROW = NH * (HD + 1)  # 1040: per-seq-chunk V row: 16 x (64 V cols + ones col)
N_CORES = 8

SCALE = 1.0 / float(np.sqrt(HD))


@functools.lru_cache(maxsize=None)
def _build(has_bv: bool):
    import concourse.bass as bass
    import concourse.tile as tile
    from concourse import bacc, mybir
    from contextlib import ExitStack

    fp32 = mybir.dt.float32
    bf16 = mybir.dt.bfloat16
    EXP = mybir.ActivationFunctionType.Exp

    nc = bacc.Bacc("TRN2", target_bir_lowering=False)

    xT = nc.dram_tensor("xT", [HID, S], bf16, kind="ExternalInput")
    wq = nc.dram_tensor("wq", [HID, HID], bf16, kind="ExternalInput")
    wk = nc.dram_tensor("wk", [HID, HID], bf16, kind="ExternalInput")
    wv = nc.dram_tensor("wv", [HID, HID], bf16, kind="ExternalInput")
    bq = nc.dram_tensor("bq", [P, NCH], fp32, kind="ExternalInput")
    bk = nc.dram_tensor("bk", [P, NCH], fp32, kind="ExternalInput")
    bv = nc.dram_tensor("bv", [HID], fp32, kind="ExternalInput") if has_bv else None
    mask = nc.dram_tensor("mask", [P, NKT], fp32, kind="ExternalInput")
    out = nc.dram_tensor("out", [S, HID], fp32, kind="ExternalOutput")

    with tile.TileContext(nc) as tc, ExitStack() as ctx:
        persist = ctx.enter_context(tc.tile_pool(name="persist", bufs=1))
        misc = ctx.enter_context(tc.tile_pool(name="misc", bufs=8))
        pT_pool = ctx.enter_context(tc.tile_pool(name="pT", bufs=3))
        out_pool = ctx.enter_context(tc.tile_pool(name="out", bufs=2))
        qkv_ps = ctx.enter_context(tc.tile_pool(name="qkv_ps", bufs=2, space="PSUM"))
        sc_ps = ctx.enter_context(tc.tile_pool(name="sc_ps", bufs=2, space="PSUM"))
        cx_ps = ctx.enter_context(tc.tile_pool(name="cx_ps", bufs=2, space="PSUM"))

        # ---- persistent SBUF tensors ----
        # per-chunk tiles: a matmul touching chunk kc then only depends on
        # that one chunk's DMA, so PE work starts ~2 chunks into the fill
        xT_c = [persist.tile([P, S], bf16, name=f"xT{c}") for c in range(NCH)]
        wq_c = [persist.tile([P, HID], bf16, name=f"wq{c}") for c in range(NCH)]
        wk_c = [persist.tile([P, HID], bf16, name=f"wk{c}") for c in range(NCH)]
        wv_c = [persist.tile([P, HID], bf16, name=f"wv{c}") for c in range(NCH)]
        qT_sb = persist.tile([P, NCH, S], bf16)  # [p, hidout_chunk, seq]
        # K^T stored zero-padded to K=128 per head: variant v holds head
        # 2c+v's 64 rows at partition offset 64*v, the other half zero.
        # This keeps the score matmuls at 128 contraction rows (FWL stays
        # enabled; 64-row weight loads serialize ~100ns/matmul otherwise).
        kTp_sb = persist.tile([P, NCH, 2, S], bf16)
        v_sb = persist.tile([P, NKT, VROW], bf16)  # [p(seq), seq_chunk, 16*(64+1)]
        bq_sb = persist.tile([P, NCH], fp32)
        bk_sb = persist.tile([P, NCH], fp32)
        mask_sb = persist.tile([P, NKT], fp32)
        bv_sb = persist.tile([P, HID], fp32, name="bv_sb") if has_bv else None

        # ---- input DMAs ----
        nc.sync.dma_start(out=bq_sb, in_=bq[:, :])
        nc.sync.dma_start(out=bk_sb, in_=bk[:, :])
        nc.sync.dma_start(out=mask_sb, in_=mask[:, :])
        if has_bv:
            # broadcast bv[HID] across all 128 partitions
            bv_bcast = bass.AP(tensor=bv.tensor if hasattr(bv, "tensor") else bv,
                               offset=0, ap=[[0, P], [1, HID]])
            nc.sync.dma_start(out=bv_sb, in_=bv_bcast)
        for c in range(NCH):
            nc.sync.dma_start(out=xT_c[c], in_=xT[c * P:(c + 1) * P, :])
            nc.sync.dma_start(out=wv_c[c], in_=wv[c * P:(c + 1) * P, :])
        for c in range(NCH):
            nc.sync.dma_start(out=wq_c[c], in_=wq[c * P:(c + 1) * P, :])
            nc.sync.dma_start(out=wk_c[c], in_=wk[c * P:(c + 1) * P, :])

        # ones columns for the softmax denominator live at col 64 of each
        # 65-wide head block; V copies below only overwrite cols 0..63
        nc.gpsimd.memset(v_sb, 1.0)
        # zero the padded K^T store on the otherwise-idle gpsimd engine;
        # the K copies later fill in only each variant's live 64 rows
        nc.gpsimd.memset(kTp_sb, 0.0)

        # warmup matmuls on scratch data while the input DMAs stream in:
        # keeps the PE busy so the HAM clock-gate reaches 8/8 before real
        # work arrives (otherwise the first ~3.4us of matmuls run at
        # 1.2GHz). Writes rotate through the score-psum slots, which see
        # no real use until well after the fill.
        wscr = persist.tile([P, 512], bf16, name="warm_scratch")
        nc.vector.memset(wscr, 0.5)
        for _ in range(16):
            wps = sc_ps.tile([P, S], fp32, name="score_psum")
            nc.tensor.matmul(
                wps[:, 0:512],
                lhsT=wscr[:, 0:P],
                rhs=wscr,
                start=True,
                stop=True,
            )

        # ---- V = X @ Wv  (+bv), stored [seq, head-interleaved 65] ----
        for st in range(NKT):  # seq chunk
            for half in range(2):
                ps = qkv_ps.tile([P, 512], fp32, name="qkv_psum")
                for kc in range(NCH):
                    nc.tensor.matmul(
                        ps,
                        lhsT=xT_c[kc][:, st * P:(st + 1) * P],
                        rhs=wv_c[kc][:, half * 512:(half + 1) * 512],
                        start=(kc == 0),
                        stop=(kc == NCH - 1),
                    )
                dst = (
                    v_sb[:, st, :]
                    .rearrange("p (h x) -> p h x", x=HD + 1)[:, half * 8:(half + 1) * 8, 0:HD]
                )
                src = ps.rearrange("p (h x) -> p h x", x=HD)
                if has_bv:
                    bvs = (
                        bv_sb[:, half * 512:(half + 1) * 512]
                        .rearrange("p (h x) -> p h x", x=HD)
                    )
                    nc.vector.tensor_add(out=dst, in0=src, in1=bvs)
                else:
                    nc.vector.tensor_copy(out=dst, in_=src)

        # ---- per hid_out chunk c: Q^T, K^T, then heads 2c, 2c+1 ----
        for c in range(NCH):
            for half in range(2):
                ps = qkv_ps.tile([P, 512], fp32, name="qkv_psum")
                for kc in range(NCH):
                    nc.tensor.matmul(
                        ps,
                        lhsT=wq_c[kc][:, c * P:(c + 1) * P],
                        rhs=xT_c[kc][:, half * 512:(half + 1) * 512],
                        start=(kc == 0),
                        stop=(kc == NCH - 1),
                    )
                nc.vector.tensor_scalar_add(
                    out=qT_sb[:, c, half * 512:(half + 1) * 512],
                    in0=ps,
                    scalar1=bq_sb[:, c:c + 1],
                )
            for half in range(2):
                ps = qkv_ps.tile([P, 512], fp32, name="qkv_psum")
                for kc in range(NCH):
                    nc.tensor.matmul(
                        ps,
                        lhsT=wk_c[kc][:, c * P:(c + 1) * P],
                        rhs=xT_c[kc][:, half * 512:(half + 1) * 512],
                        start=(kc == 0),
                        stop=(kc == NCH - 1),
                    )
                for sub in range(2):  # head 2c+sub lives at partitions 64*sub..
                    po = 64 * sub
                    nc.vector.tensor_scalar_add(
                        out=kTp_sb[po:po + HD, c, sub, half * 512:(half + 1) * 512],
                        in0=ps[po:po + HD, :],
                        scalar1=bk_sb[po:po + HD, c:c + 1],
                    )

            # ---- attention for the two heads living in chunk c ----
            pT_tiles = []
            for sub in range(2):
                h = 2 * c + sub
                pT_h = pT_pool.tile([P, NKT, S], bf16, name="pT")
                pT_tiles.append(pT_h)
                for kt in range(NKT):
                    ps = sc_ps.tile([P, S], fp32, name="score_psum")
                    for half in range(2):
                        nc.tensor.matmul(
                            ps[:, half * 512:(half + 1) * 512],
                            lhsT=kTp_sb[:, c, sub, kt * P:(kt + 1) * P],
                            rhs=qT_sb[:, c, half * 512:(half + 1) * 512],
                            start=True,
                            stop=True,
                        )
                    # P^T = exp(scores/8 + mask_k); bf16 out, straight to SBUF
                    nc.scalar.activation(
                        out=pT_h[:, kt, :],
                        in_=ps,
                        func=EXP,
                        bias=mask_sb[:, kt:kt + 1],
                        scale=SCALE,
                    )

            for sub in range(2):
                h = 2 * c + sub
                pT_h = pT_tiles[sub]
                head_out = out_pool.tile([P, NQT, HD], fp32, name="head_out")
                for qt in range(NQT):
                    cps = cx_ps.tile([P, HD + 1], fp32, name="ctx_psum")
                    for kc in range(NKT):
                        nc.tensor.matmul(
                            cps,
                            lhsT=pT_h[:, kc, qt * P:(qt + 1) * P],
                            rhs=v_sb[:, kc, h * (HD + 1):(h + 1) * (HD + 1)],
                            start=(kc == 0),
                            stop=(kc == NKT - 1),
                        )
                    recip = misc.tile([P, 1], fp32, name="recip")
                    nc.vector.reciprocal(recip, cps[:, HD:HD + 1])
                    nc.vector.tensor_scalar_mul(
                        out=head_out[:, qt, :],
                        in0=cps[:, 0:HD],
                        scalar1=recip,
                    )
                # stream this head's output columns out while later heads run
                for qt in range(NQT):
                    nc.sync.dma_start(
                        out=out[qt * P:(qt + 1) * P, h * HD:(h + 1) * HD],
                        in_=head_out[:, qt, :],
                    )

    nc.finalize()
    return nc


def _prep_inputs(inputs):
    bf16 = ml_dtypes.bfloat16
    hs = np.asarray(inputs["hidden_states"], dtype=np.float32)
    am = np.asarray(inputs["attention_mask"], dtype=np.float32)
    Wq = np.asarray(inputs["Wq"], dtype=np.float32)
    Wk = np.asarray(inputs["Wk"], dtype=np.float32)
    Wv = np.asarray(inputs["Wv"], dtype=np.float32)
    bq = np.asarray(inputs["bq"], dtype=np.float32)
    bk = np.asarray(inputs["bk"], dtype=np.float32)
    bv = np.asarray(inputs["bv"], dtype=np.float32)

    has_bv = bool(np.any(bv))

    wq_b = np.ascontiguousarray(Wq.astype(bf16))
    wk_b = np.ascontiguousarray(Wk.astype(bf16))
    wv_b = np.ascontiguousarray(Wv.astype(bf16))
    bq_c = np.ascontiguousarray(bq.reshape(NCH, P).T)
    bk_c = np.ascontiguousarray(bk.reshape(NCH, P).T)

    hs_b = hs.astype(bf16)
    in_maps = []
    for b in range(B):
        m = {
            "xT": np.ascontiguousarray(hs_b[b].T),
            "wq": wq_b,
            "wk": wk_b,
            "wv": wv_b,
            "bq": bq_c,
            "bk": bk_c,
            "mask": np.ascontiguousarray(am[b, 0, 0].reshape(NKT, P).T),
        }
        if has_bv:
            m["bv"] = bv
        in_maps.append(m)
    return in_maps, has_bv


def _run(inputs, trace=False):
    from concourse.bass_utils import run_bass_kernel_spmd

    in_maps, has_bv = _prep_inputs(inputs)
    nc = _build(has_bv)
    res = run_bass_kernel_spmd(
        nc, in_maps, core_ids=list(range(N_CORES)), trace=trace
    )
    out = np.stack([np.asarray(r["out"], dtype=np.float32) for r in res.results])
    return out, res


def kernel(**inputs) -> np.ndarray:
    out, _ = _run(inputs, trace=False)
    return out


# revision 20
# speedup vs baseline: 1.0066x; 1.0066x over previous
"""BERT self-attention on 8 Trainium2 NeuronCores.

Sharding: data-parallel over batch (B=8 -> 1 batch element per core).
Every core runs the same single-core Bass kernel on its own batch slice;
weights/mask are replicated. The final output is a host-side stack.

Per-core algorithm (S=1024, HID=1024, NH=16, HD=64), all matmuls bf16
with fp32 PSUM accumulation:

  xT = X^T (host-transposed, bf16)             [HID, S]
  Q^T = Wq^T @ X^T   (lhsT = Wq natural)       [HID, S]  (+bq per-partition)
  K^T = Wk^T @ X^T                             [HID, S]  (+bk per-partition)
  V   = X @ Wv       (lhsT = xT)               [S, HID]  (+bv broadcast; see below)
  per head h:
    S^T = K_h @ Q_h^T            (scoresT: [k, q]; K^T stored zero-padded to
                                  128 contraction rows so FWL stays enabled)
    P^T = exp(S^T/8 + mask[k])   (ScalarE, mask is per-partition in this layout,
                                  max-subtraction skipped: |scores/8| <~ 4)
    ctx = P^T.T @ [V_h + bv | 1] (lhsT = P^T directly, no transposes anywhere;
                                  the ones column yields the softmax denominator Z)
    out[:, h] = ctx[:, :64] * (1/Z)   (== softmax(S) @ (V+bv) = attn + bv)

  Measured on 8 axon TRN2 cores: ~215 us/core HW exec, rel err 3.5e-3 vs
  the fp32 reference (all error from bf16 matmul operands).
"""

import functools

import numpy as np
import ml_dtypes

B, S, HID = 8, 1024, 1024
NH, HD = 16, 64
P = 128
NCH = HID // P  # hid chunks (8)
NKT = S // P  # key tiles (8)
NQT = S // P  # query tiles (8)
VROW = NH * (HD + 1)  # 1040: per-seq-chunk V row: 16 x (64 V cols + ones col)
N_CORES = 8

SCALE = 1.0 / float(np.sqrt(HD))


@functools.lru_cache(maxsize=None)
def _build(has_bv: bool):
    import concourse.bass as bass
    import concourse.tile as tile
    from concourse import bacc, mybir
    from contextlib import ExitStack

    fp32 = mybir.dt.float32
    bf16 = mybir.dt.bfloat16
    EXP = mybir.ActivationFunctionType.Exp

    nc = bacc.Bacc("TRN2", target_bir_lowering=False)

    xT = nc.dram_tensor("xT", [HID, S], bf16, kind="ExternalInput")
    wq = nc.dram_tensor("wq", [HID, HID], bf16, kind="ExternalInput")
    wk = nc.dram_tensor("wk", [HID, HID], bf16, kind="ExternalInput")
    wv = nc.dram_tensor("wv", [HID, HID], bf16, kind="ExternalInput")
    bq = nc.dram_tensor("bq", [P, NCH], fp32, kind="ExternalInput")
    bk = nc.dram_tensor("bk", [P, NCH], fp32, kind="ExternalInput")
    bv = nc.dram_tensor("bv", [HID], fp32, kind="ExternalInput") if has_bv else None
    mask = nc.dram_tensor("mask", [P, NKT], fp32, kind="ExternalInput")
    out = nc.dram_tensor("out", [S, HID], fp32, kind="ExternalOutput")

    with tile.TileContext(nc) as tc, ExitStack() as ctx:
        persist = ctx.enter_context(tc.tile_pool(name="persist", bufs=1))
        misc = ctx.enter_context(tc.tile_pool(name="misc", bufs=8))
        pT_pool = ctx.enter_context(tc.tile_pool(name="pT", bufs=3))
        out_pool = ctx.enter_context(tc.tile_pool(name="out", bufs=2))
        qkv_ps = ctx.enter_context(tc.tile_pool(name="qkv_ps", bufs=2, space="PSUM"))
        sc_ps = ctx.enter_context(tc.tile_pool(name="sc_ps", bufs=2, space="PSUM"))
        cx_ps = ctx.enter_context(tc.tile_pool(name="cx_ps", bufs=2, space="PSUM"))

        # ---- persistent SBUF tensors ----
        # per-chunk tiles: a matmul touching chunk kc then only depends on
        # that one chunk's DMA, so PE work starts ~2 chunks into the fill
        xT_c = [persist.tile([P, S], bf16, name=f"xT{c}") for c in range(NCH)]
        wq_c = [persist.tile([P, HID], bf16, name=f"wq{c}") for c in range(NCH)]
        wk_c = [persist.tile([P, HID], bf16, name=f"wk{c}") for c in range(NCH)]
        wv_c = [persist.tile([P, HID], bf16, name=f"wv{c}") for c in range(NCH)]
        qT_sb = persist.tile([P, NCH, S], bf16)  # [p, hidout_chunk, seq]
        # K^T stored zero-padded to K=128 per head: variant v holds head
        # 2c+v's 64 rows at partition offset 64*v, the other half zero.
        # This keeps the score matmuls at 128 contraction rows (FWL stays
        # enabled; 64-row weight loads serialize ~100ns/matmul otherwise).
        kTp_sb = persist.tile([P, NCH, 2, S], bf16)
        v_sb = persist.tile([P, NKT, VROW], bf16)  # [p(seq), seq_chunk, 16*(64+1)]
        bq_sb = persist.tile([P, NCH], fp32)
        bk_sb = persist.tile([P, NCH], fp32)
        mask_sb = persist.tile([P, NKT], fp32)
        bv_sb = persist.tile([P, HID], fp32, name="bv_sb") if has_bv else None

        # ---- input DMAs ----
        nc.sync.dma_start(out=bq_sb, in_=bq[:, :])
        nc.sync.dma_start(out=bk_sb, in_=bk[:, :])
        nc.sync.dma_start(out=mask_sb, in_=mask[:, :])
        if has_bv:
            # broadcast bv[HID] across all 128 partitions
            bv_bcast = bass.AP(tensor=bv.tensor if hasattr(bv, "tensor") else bv,
                               offset=0, ap=[[0, P], [1, HID]])
            nc.sync.dma_start(out=bv_sb, in_=bv_bcast)
        for c in range(NCH):
            nc.sync.dma_start(out=xT_c[c], in_=xT[c * P:(c + 1) * P, :])
            nc.sync.dma_start(out=wv_c[c], in_=wv[c * P:(c + 1) * P, :])
        for c in range(NCH):
            nc.sync.dma_start(out=wq_c[c], in_=wq[c * P:(c + 1) * P, :])
            nc.sync.dma_start(out=wk_c[c], in_=wk[c * P:(c + 1) * P, :])

        # ones columns for the softmax denominator live at col 64 of each
        # 65-wide head block; V copies below only overwrite cols 0..63
        nc.gpsimd.memset(v_sb, 1.0)
        # zero the padded K^T store on the otherwise-idle gpsimd engine;
        # the K copies later fill in only each variant's live 64 rows
        nc.gpsimd.memset(kTp_sb, 0.0)

        # warmup matmuls on scratch data while the input DMAs stream in:
        # keeps the PE busy so the HAM clock-gate reaches 8/8 before real
        # work arrives (otherwise the first ~3.4us of matmuls run at
        # 1.2GHz). Writes rotate through the score-psum slots, which see
        # no real use until well after the fill.
        wscr = persist.tile([P, 512], bf16, name="warm_scratch")
        nc.vector.memset(wscr, 0.5)
        for _ in range(16):
            wps = sc_ps.tile([P, S], fp32, name="score_psum")
            nc.tensor.matmul(
                wps[:, 0:512],
                lhsT=wscr[:, 0:P],
                rhs=wscr,
                start=True,
                stop=True,
            )

        # ---- V = X @ Wv  (+bv), stored [seq, head-interleaved 65] ----
        for st in range(NKT):  # seq chunk
            for half in range(2):
                ps = qkv_ps.tile([P, 512], fp32, name="qkv_psum")
                for kc in range(NCH):
                    nc.tensor.matmul(
                        ps,
                        lhsT=xT_c[kc][:, st * P:(st + 1) * P],
                        rhs=wv_c[kc][:, half * 512:(half + 1) * 512],
                        start=(kc == 0),
                        stop=(kc == NCH - 1),
                    )
                dst = (
                    v_sb[:, st, :]
                    .rearrange("p (h x) -> p h x", x=HD + 1)[:, half * 8:(half + 1) * 8, 0:HD]
                )
                src = ps.rearrange("p (h x) -> p h x", x=HD)
                if has_bv:
                    bvs = (
                        bv_sb[:, half * 512:(half + 1) * 512]
                        .rearrange("p (h x) -> p h x", x=HD)
                    )
                    nc.vector.tensor_add(out=dst, in0=src, in1=bvs)
                else:
                    nc.vector.tensor_copy(out=dst, in_=src)

        # ---- per hid_out chunk c: Q^T, K^T, then heads 2c, 2c+1 ----
        for c in range(NCH):
            for half in range(2):
                ps = qkv_ps.tile([P, 512], fp32, name="qkv_psum")
                for kc in range(NCH):
                    nc.tensor.matmul(
                        ps,
                        lhsT=wq_c[kc][:, c * P:(c + 1) * P],
                        rhs=xT_c[kc][:, half * 512:(half + 1) * 512],
                        start=(kc == 0),
                        stop=(kc == NCH - 1),
                    )
                nc.vector.tensor_scalar_add(
                    out=qT_sb[:, c, half * 512:(half + 1) * 512],
                    in0=ps,
                    scalar1=bq_sb[:, c:c + 1],
                )
            for half in range(2):
                ps = qkv_ps.tile([P, 512], fp32, name="qkv_psum")
                for kc in range(NCH):
                    nc.tensor.matmul(
                        ps,
                        lhsT=wk_c[kc][:, c * P:(c + 1) * P],
                        rhs=xT_c[kc][:, half * 512:(half + 1) * 512],
                        start=(kc == 0),
                        stop=(kc == NCH - 1),
                    )
                for sub in range(2):  # head 2c+sub lives at partitions 64*sub..
                    po = 64 * sub
                    nc.vector.tensor_scalar_add(
                        out=kTp_sb[po:po + HD, c, sub, half * 512:(half + 1) * 512],
                        in0=ps[po:po + HD, :],
                        scalar1=bk_sb[po:po + HD, c:c + 1],
                    )

            # ---- attention for the two heads living in chunk c ----
            pT_tiles = []
            for sub in range(2):
                h = 2 * c + sub
                pT_h = pT_pool.tile([P, NKT, S], bf16, name="pT")
                pT_tiles.append(pT_h)
                for kt in range(NKT):
                    ps = sc_ps.tile([P, S], fp32, name="score_psum")
                    for half in range(2):
                        nc.tensor.matmul(
                            ps[:, half * 512:(half + 1) * 512],
                            lhsT=kTp_sb[:, c, sub, kt * P:(kt + 1) * P],
                            rhs=qT_sb[:, c, half * 512:(half + 1) * 512],
                            start=True,
                            stop=True,
                        )
                    # P^T = exp(scores/8 + mask_k); bf16 out, straight to SBUF
                    nc.scalar.activation(
                        out=pT_h[:, kt, :],
                        in_=ps,
                        func=EXP,
                        bias=mask_sb[:, kt:kt + 1],
                        scale=SCALE,
                    )

            for sub in range(2):
                h = 2 * c + sub
                pT_h = pT_tiles[sub]
                head_out = out_pool.tile([P, NQT, HD], fp32, name="head_out")
                for qt in range(NQT):
                    cps = cx_ps.tile([P, HD + 1], fp32, name="ctx_psum")
                    for kc in range(NKT):
                        nc.tensor.matmul(
                            cps,
                            lhsT=pT_h[:, kc, qt * P:(qt + 1) * P],
                            rhs=v_sb[:, kc, h * (HD + 1):(h + 1) * (HD + 1)],
                            start=(kc == 0),
                            stop=(kc == NKT - 1),
                        )
                    recip = misc.tile([P, 1], fp32, name="recip")
                    nc.vector.reciprocal(recip, cps[:, HD:HD + 1])
                    nc.vector.tensor_scalar_mul(
                        out=head_out[:, qt, :],
                        in0=cps[:, 0:HD],
                        scalar1=recip,
                    )
                # stream this head's output columns out while later heads run
                for qt in range(NQT):
                    nc.sync.dma_start(
                        out=out[qt * P:(qt + 1) * P, h * HD:(h + 1) * HD],
                        in_=head_out[:, qt, :],
                    )

    nc.finalize()
    return nc


def _prep_inputs(inputs):
    bf16 = ml_dtypes.bfloat16
    hs = np.asarray(inputs["hidden_states"], dtype=np.float32)
    am = np.asarray(inputs["attention_mask"], dtype=np.float32)
    Wq = np.asarray(inputs["Wq"], dtype=np.float32)
    Wk = np.asarray(inputs["Wk"], dtype=np.float32)
    Wv = np.asarray(inputs["Wv"], dtype=np.float32)
    bq = np.asarray(inputs["bq"], dtype=np.float32)
    bk = np.asarray(inputs["bk"], dtype=np.float32)
    bv = np.asarray(inputs["bv"], dtype=np.float32)

    has_bv = bool(np.any(bv))

    wq_b = np.ascontiguousarray(Wq.astype(bf16))
    wk_b = np.ascontiguousarray(Wk.astype(bf16))
    wv_b = np.ascontiguousarray(Wv.astype(bf16))
    bq_c = np.ascontiguousarray(bq.reshape(NCH, P).T)
    bk_c = np.ascontiguousarray(bk.reshape(NCH, P).T)

    hs_b = hs.astype(bf16)
    in_maps = []
    for b in range(B):
        m = {
            "xT": np.ascontiguousarray(hs_b[b].T),
            "wq": wq_b,
            "wk": wk_b,
            "wv": wv_b,
            "bq": bq_c,
            "bk": bk_c,
            "mask": np.ascontiguousarray(am[b, 0, 0].reshape(NKT, P).T),
        }
        if has_bv:
            m["bv"] = bv
        in_maps.append(m)
    return in_maps, has_bv


def _run(inputs, trace=False, trace_cores=None):
    from concourse.bass_utils import run_bass_kernel_spmd

    in_maps, has_bv = _prep_inputs(inputs)
    nc = _build(has_bv)
    res = run_bass_kernel_spmd(
        nc, in_maps, core_ids=list(range(N_CORES)), trace=trace,
        trace_cores=trace_cores,
    )
    out = np.stack([np.asarray(r["out"], dtype=np.float32) for r in res.results])
    return out, res


def kernel(**inputs) -> np.ndarray:
    out, _ = _run(inputs, trace=False)
    return out
